# revision 1
# baseline (speedup 1.0000x reference)
"""Enformer-style relative-position attention (nn_Attention_27925877358942) for
8 Trainium2 NeuronCores.

Contract: kernel(**inputs) takes the FULL unsharded inputs (keys as in
setup_inputs()) and returns the full [1, 4096, 1536] float32 output.

Sharding: one head per core (8 heads / 8 cores). Host precomputes the
deterministic positional-feature table and x^T in fp16, slices per-head
weights, runs the SPMD Bass kernel via run_bass_kernel_spmd, and sums the
per-head output projections (+ b_out).

Device pipeline per core (head h), N=4096, d=64:
  - q^T,k^T (fp16, [64,N]) and v ([N,65] with ones col) projections on PE
  - r^T = (pos @ Wrelk_h)^T from the positional table
  - per query tile I: window logits em = exp((q+bp) . r[t0:t0+4223]) (ACT, bf16)
  - relative_shift via DRAM roundtrip: sheared strided read
      shr[di, j] = em[di, 127-di+j] (partition step = rowpitch-1 elements)
  - content logits transposed C^T = k_J . q_I (PE), exp on ACT
  - pT = exp(C^T) * transpose(shr) (PE transpose + DVE multiply, bf16)
  - O = pT.T @ [v|1] accumulated in PSUM; epilogue normalizes by the row sums
    and applies the per-head slice of W_out; host sums partials over heads.

This walrus build accepts at most ONE sync wait per instruction, so after
Tile scheduling every multi-wait instruction is split by inserting
wait-carrying NoOps just before it on the same engine (split_multi_waits),
and the Tile tail drain is built with the same constraint.
"""


_DRAIN_PATCHED = [False]


def _patch_tile_drain():
    if _DRAIN_PATCHED[0]:
        return
    _DRAIN_PATCHED[0] = True
    import concourse.tile as tile_mod
    from concourse.vector_clock import ScopedClock

    MAX_WAITS = 1

    def _drain_and_barrier(self, tick_clock, wait_clock):
        nc = self.nc
        drain_inst = nc.sync.drain()
        wait_clock.add_sem_waits(drain_inst.ins, ScopedClock({None: tick_clock.global_clock}))
        si = drain_inst.ins.sync_info
        waits = list(si.on_wait) if si is not None and si.on_wait else []
        if len(waits) > MAX_WAITS:
            si.on_wait = waits[:MAX_WAITS]
            rest = waits[MAX_WAITS:]
            import concourse.mybir as _mb
            for i in range(0, len(rest), MAX_WAITS):
                extra = nc.sync.drain()
                esi = extra.ins.sync_info
                if esi is None:
                    extra.ins.sync_info = _mb.SyncInfo(on_wait=rest[i:i + MAX_WAITS], on_update=[])
                else:
                    esi.on_wait = rest[i:i + MAX_WAITS]
        nc.all_engine_barrier()
        assert self.sems is not None
        popped = nc._tile_sem_poison_stack.pop()
        assert popped is self._sem_poison
        nc.clear_and_free_semaphores(list(self.sems.allocated().values()))
        nc.all_engine_barrier()

    tile_mod.TileContext._drain_and_barrier = _drain_and_barrier


def split_multi_waits(nc):
    """This walrus build allows at most ONE sync wait per instruction.
    Move extra waits onto InstNoOp carriers inserted just before, on the
    same engine queue (sequencers execute in order, so semantics hold)."""
    import concourse.mybir as mb
    n_split = 0
    for fn in nc.m.functions:
        for bb in fn.blocks:
            insts = list(bb.instructions)
            out = []
            for inst in insts:
                si = inst.sync_info
                waits = list(si.on_wait) if si is not None and si.on_wait else []
                if len(waits) > 1:
                    for w in waits[:-1]:
                        n_split += 1
                        nop = mb.InstNoOp(
                            name=f"waitsplit-{n_split}",
                            engine=inst.engine,
                            sync_info=mb.SyncInfo(on_wait=[w], on_update=[]),
                        )
                        out.append(nop)
                    si.on_wait = [waits[-1]]
                out.append(inst)
            if len(out) != len(insts):
                bb.instructions[:] = out
    return n_split


import math
from contextlib import ExitStack

import numpy as np

import concourse.bass as bass
import concourse.tile as tile
from concourse import mybir
from concourse.bass import ts, ds
from concourse.masks import make_identity

F32 = mybir.dt.float32
BF16 = mybir.dt.bfloat16
FP16 = mybir.dt.float16
AF = mybir.ActivationFunctionType

DIM = 1536
H = 8
D = 64


def build(N, split_waits=True):
    Q = N // 128           # query tiles
    NJ = N // 128          # key tiles
    PW = 2 * N             # padded positional width (2N-1 real cols + 1 pad)
    WN = N + 128           # rel window width per q-tile (incl. 1 pad col)
    KD = DIM // 128        # contraction tiles for projections

    nc = bass.Bass("TRN2", target_bir_lowering=False, debug=False)

    xT_d = nc.dram_tensor("xT", [DIM, N], FP16, kind="ExternalInput")
    posT_d = nc.dram_tensor("posT", [192, PW], FP16, kind="ExternalInput")
    wq_d = nc.dram_tensor("wq", [DIM, D], FP16, kind="ExternalInput")
    wk_d = nc.dram_tensor("wk", [DIM, D], FP16, kind="ExternalInput")
    wv_d = nc.dram_tensor("wv", [DIM, D], FP16, kind="ExternalInput")
    wrk_d = nc.dram_tensor("wrk", [192, D], FP16, kind="ExternalInput")
    wo_d = nc.dram_tensor("wo", [D, DIM], BF16, kind="ExternalInput")
    bc_d = nc.dram_tensor("bc", [D, 1], F32, kind="ExternalInput")
    bp_d = nc.dram_tensor("bp", [D, 1], F32, kind="ExternalInput")
    out_d = nc.dram_tensor("out", [N, DIM], FP16, kind="ExternalOutput")
    em_d = nc.dram_tensor("em_scratch", [Q * 128, WN], BF16, kind="Internal")

    scale = D ** -0.5

    with tile.TileContext(nc) as tc, ExitStack() as ctx:
        consts = ctx.enter_context(tc.tile_pool(name="consts", bufs=1))
        persist = ctx.enter_context(tc.tile_pool(name="persist", bufs=1))

        # ---- constants ----
        ident = consts.tile([128, 128], BF16, tag="ident")
        make_identity(nc, ident[:])
        bc_sb = consts.tile([D, 1], F32, tag="bc")
        nc.sync.dma_start(out=bc_sb[:], in_=bc_d.ap())
        bp_sb = consts.tile([D, 1], F32, tag="bp")
        nc.sync.dma_start(out=bp_sb[:], in_=bp_d.ap())
        wo_sb = consts.tile([D, DIM], BF16, tag="wo")
        nc.sync.dma_start(out=wo_sb[:], in_=wo_d.ap())

        wqk_sb = consts.tile([128, KD, 2 * D], FP16, tag="wqk")
        wv_sb = consts.tile([128, KD, D], FP16, tag="wv")
        nc.sync.dma_start(out=wqk_sb[:, :, 0:D],
                          in_=wq_d.ap().rearrange("(t p) c -> p t c", p=128))
        nc.sync.dma_start(out=wqk_sb[:, :, D:2 * D],
                          in_=wk_d.ap().rearrange("(t p) c -> p t c", p=128))
        nc.sync.dma_start(out=wv_sb[:],
                          in_=wv_d.ap().rearrange("(t p) c -> p t c", p=128))
        wrk_sb = consts.tile([96, 2, D], FP16, tag="wrk")
        for u in range(2):
            nc.sync.dma_start(out=wrk_sb[:, u, :], in_=wrk_d[ts(u, 96), :])

        # ---- persistent activations ----
        qcT = persist.tile([D, N], FP16, tag="qcT")
        qpT = persist.tile([D, N], FP16, tag="qpT")
        kT = persist.tile([D, N], FP16, tag="kT")
        rT = persist.tile([D, PW], FP16, tag="rT")
        vext = persist.tile([128, NJ * (D + 1)], BF16, tag="vext")

        # ---- phases 1-2: rel-k table + projections ----
        with tc.tile_pool(name="stream", bufs=1) as stream, \
             tc.tile_pool(name="prep_psum", bufs=2, space="PSUM") as prep_psum:
            # rel-k table first: independent of x, runs during the xT load
            pall = stream.tile([96, 2, PW], FP16, tag="pall")
            nc.sync.dma_start(out=pall[:, 0, :], in_=posT_d[0:96, :])
            nc.sync.dma_start(out=pall[:, 1, :], in_=posT_d[96:192, :])
            for rc in reversed(range(PW // 512)):
                c0 = rc * 512
                pc = pall[:, :, ds(c0, 512)]
                ps_r = prep_psum.tile([D, 512], F32, tag="ps_qk")
                for u in range(2):
                    nc.tensor.matmul(
                        ps_r[:], wrk_sb[:, u, :], pc[:, u, :],
                        start=(u == 0), stop=(u == 1),
                    )
                nc.scalar.copy(out=rT[:, ds(c0, 512)], in_=ps_r[:])

            xall = stream.tile([128, KD, N], FP16, tag="xall")
            xT_v = xT_d.ap().rearrange("(t p) n -> p t n", p=128)
            for oct_ in range(8):
                h0 = oct_ * (N // 8)
                nc.sync.dma_start(
                    out=xall[:, :, ds(h0, N // 8)],
                    in_=xT_v[:, :, ds(h0, N // 8)],
                )
            for ic in range(N // 512):
                i0 = ic * 512
                xc = xall[:, :, ds(i0, 512)]
                ps_qk = prep_psum.tile([128, 512], F32, tag="ps_qk")
                for kd in range(KD):
                    nc.tensor.matmul(
                        ps_qk[:], wqk_sb[:, kd, :], xc[:, kd, :],
                        start=(kd == 0), stop=(kd == KD - 1),
                    )
                nc.scalar.activation(
                    out=qcT[:, ds(i0, 512)], in_=ps_qk[0:D, :], func=AF.Identity,
                    bias=bc_sb[:], scale=scale,
                )
                nc.scalar.activation(
                    out=qpT[:, ds(i0, 512)], in_=ps_qk[0:D, :], func=AF.Identity,
                    bias=bp_sb[:], scale=scale,
                )
                nc.scalar.copy(out=kT[:, ds(i0, 512)], in_=ps_qk[D:2 * D, :])
                for isb in range(4):
                    J = ic * 4 + isb
                    ps_v = prep_psum.tile([128, D], F32, tag="ps_v")
                    for kd in range(KD):
                        nc.tensor.matmul(
                            ps_v[:], xc[:, kd, ts(isb, 128)], wv_sb[:, kd, :],
                            start=(kd == 0), stop=(kd == KD - 1),
                        )
                    nc.scalar.copy(out=vext[:, ds(J * (D + 1), D)], in_=ps_v[:])
                    nc.vector.memset(vext[:, ds(J * (D + 1) + D, 1)], 1.0)

        # ---- phase 3: main loop, q-tiles in pairs ----
        work = ctx.enter_context(tc.tile_pool(name="work", bufs=2))
        wshear = ctx.enter_context(tc.tile_pool(name="wshear", bufs=4))
        sm = ctx.enter_context(tc.tile_pool(name="sm", bufs=3))
        ppool_m = ctx.enter_context(tc.tile_pool(name="ppool_m", bufs=2, space="PSUM"))
        ppool_ct = ctx.enter_context(tc.tile_pool(name="ppool_ct", bufs=2, space="PSUM"))
        ppool_st = ctx.enter_context(tc.tile_pool(name="ppool_st", bufs=1, space="PSUM"))
        ppool_epi = ctx.enter_context(tc.tile_pool(name="ppool_epi", bufs=1, space="PSUM"))

        for g in range(Q // 2):
            i0g = g * 256
            shr_pair = []
            for q in range(2):
                I = 2 * g + q
                i0 = I * 128
                t0 = N - 1 - i0 - 127

                em_sb = wshear.tile([128, WN], BF16, tag="em")
                n_full = (WN - 128) // 1024
                chunks = [(c * 1024, 1024) for c in range(n_full)]
                chunks.append((n_full * 1024, WN - 1 - n_full * 1024))
                for (c0, cw) in chunks:
                    ps = ppool_m.tile([128, 1024], F32, tag="ps_m")
                    for s0 in range(0, cw, 512):
                        sw = min(512, cw - s0)
                        nc.tensor.matmul(
                            ps[:, ds(s0, sw)], qpT[:, ds(i0, 128)],
                            rT[:, ds(t0 + c0 + s0, sw)],
                            start=True, stop=True,
                        )
                    nc.scalar.activation(
                        out=em_sb[:, ds(c0, cw)], in_=ps[:, 0:cw], func=AF.Exp,
                    )
                nc.sync.dma_start(out=em_d[ds(i0, 128), 0:WN - 1],
                                  in_=em_sb[:, 0:WN - 1])
                shr_sb = wshear.tile([128, N], BF16, tag="shr")
                shear_ap = bass.AP(em_d, i0 * WN + 127, [[WN - 1, 128], [1, N]])
                nc.sync.dma_start(out=shr_sb[:], in_=shear_ap)
                shr_pair.append(shr_sb)

            # content logits transposed: ecT[dj, J*256 + q*128 + di]
            ecT_sb = work.tile([128, NJ * 256], BF16, tag="ecT")
            for Jg in range(NJ // 2):
                ps = ppool_ct.tile([128, 512], F32, tag="ps_ct")
                for u in range(2):
                    J = Jg * 2 + u
                    nc.tensor.matmul(
                        ps[:, ts(u, 256)], kT[:, ts(J, 128)], qcT[:, ds(i0g, 256)],
                        start=True, stop=True,
                    )
                nc.scalar.activation(
                    out=ecT_sb[:, ds(Jg * 512, 512)], in_=ps[:], func=AF.Exp,
                )

            # pT = ecT * shr^T
            pT_sb = work.tile([128, NJ * 256], BF16, tag="pT")
            for Jg in range(NJ // 4):
                ps_t = ppool_st.tile([128, 1024], BF16, tag="ps_st")
                for u in range(4):
                    J = Jg * 4 + u
                    for q in range(2):
                        nc.tensor.transpose(
                            ps_t[:, ds(u * 256 + q * 128, 128)],
                            shr_pair[q][:, ts(J, 128)], ident[:],
                        )
                nc.vector.tensor_mul(
                    pT_sb[:, ds(Jg * 1024, 1024)], ecT_sb[:, ds(Jg * 1024, 1024)], ps_t[:]
                )

            # PV + epilogue per q-tile
            for q in range(2):
                i0 = i0g + q * 128
                ps_o = ppool_epi.tile([128, D + 1], F32, tag="ps_epi")
                for J in range(NJ):
                    nc.tensor.matmul(
                        ps_o[:], pT_sb[:, ds(J * 256 + q * 128, 128)],
                        vext[:, ds(J * (D + 1), D + 1)],
                        start=(J == 0), stop=(J == NJ - 1),
                    )
                rc_sb = sm.tile([128, 1], F32, tag="rc")
                nc.vector.reciprocal(out=rc_sb[:], in_=ps_o[:, D:D + 1])
                o_sb = sm.tile([128, D], BF16, tag="o")
                nc.vector.tensor_copy(o_sb[:], ps_o[:, 0:D])
                ps_ot = ppool_epi.tile([D, 128], BF16, tag="ps_epi")
                nc.tensor.transpose(ps_ot[:], o_sb[:], ident[:])
                otT_sb = sm.tile([D, 128], BF16, tag="otT")
                nc.vector.tensor_copy(otT_sb[:], ps_ot[:])
                out_sb = work.tile([128, DIM], FP16, tag="out")
                for w in range(DIM // 512):
                    ps_op = ppool_epi.tile([128, 512], F32, tag="ps_epi")
                    nc.tensor.matmul(
                        ps_op[:], otT_sb[:], wo_sb[:, ts(w, 512)],
                        start=True, stop=True,
                    )
                    nc.vector.tensor_scalar_mul(
                        out_sb[:, ts(w, 512)], ps_op[:], rc_sb[:]
                    )
                nc.sync.dma_start(out=out_d[ds(i0, 128), :], in_=out_sb[:])

    if split_waits:
        _patch_tile_drain()
        split_multi_waits(nc)
    return nc


# ---------------- host side ----------------

def get_positional_embed_np(seq_len, feature_size):
    distances = np.arange(-seq_len + 1, seq_len)
    nb = feature_size // 2
    pow_rate = math.exp(math.log(seq_len + 1) / nb)
    center_widths = np.power(np.float32(pow_rate), np.arange(1, nb + 1, dtype=np.float32)) - 1.0
    emb = (center_widths[None, :] > np.abs(distances)[:, None]).astype(np.float32)
    signed = np.sign(distances).astype(np.float32)[:, None] * emb
    return np.concatenate([emb, signed], axis=-1)  # [2n-1, F]


def make_in_maps(x, W_q, W_k, W_v, W_rel_k, W_out, rel_content_bias, rel_pos_bias):
    B, N, _ = np.asarray(x).shape
    PW = 2 * N
    f16 = np.float16
    import ml_dtypes
    bf16 = ml_dtypes.bfloat16
    xT = np.ascontiguousarray(np.asarray(x[0], np.float32).T).astype(f16)
    pos = get_positional_embed_np(N, np.asarray(W_rel_k).shape[0])
    posT = np.zeros((192, PW), np.float32)
    posT[:, : 2 * N - 1] = pos.T
    posT = posT.astype(f16)
    in_maps = []
    for h in range(H):
        sl = slice(h * D, (h + 1) * D)
        in_maps.append({
            "xT": xT,
            "posT": posT,
            "wq": np.ascontiguousarray(np.asarray(W_q)[:, sl]).astype(f16),
            "wk": np.ascontiguousarray(np.asarray(W_k)[:, sl]).astype(f16),
            "wv": np.ascontiguousarray(np.asarray(W_v)[:, sl]).astype(f16),
            "wrk": np.ascontiguousarray(np.asarray(W_rel_k)[:, sl]).astype(f16),
            "wo": np.ascontiguousarray(np.asarray(W_out)[sl, :]).astype(bf16),
            "bc": np.ascontiguousarray(
                np.asarray(rel_content_bias, np.float32)[0, h, 0, :].reshape(D, 1)),
            "bp": np.ascontiguousarray(
                np.asarray(rel_pos_bias, np.float32)[0, h, 0, :].reshape(D, 1)),
        })
    return in_maps


def combine_outputs(results, b_out):
    acc = None
    for r in results:
        p = r["out"].astype(np.float32)
        acc = p if acc is None else acc + p
    acc = acc + np.asarray(b_out, np.float32)[None, :]
    return acc[None]  # [1, N, DIM]


# ---------------- entry point ----------------

_NC_CACHE = {}


def kernel(x, W_q, W_k, W_v, W_rel_k, W_out, b_out,
           rel_content_bias, rel_pos_bias):
    """Full-input entry: shards per head across 8 NeuronCores, returns the
    full [1, N, 1536] float32 output."""
    from concourse import bass_utils

    x = np.asarray(x)
    N = x.shape[1]
    if N not in _NC_CACHE:
        _NC_CACHE[N] = build(N)
    nc = _NC_CACHE[N]
    in_maps = make_in_maps(x, W_q, W_k, W_v, W_rel_k, W_out,
                           rel_content_bias, rel_pos_bias)
    res = bass_utils.run_bass_kernel_spmd(nc, in_maps, core_ids=list(range(H)))
    return combine_outputs(res.results, b_out).astype(np.float32)



# revision 3
# speedup vs baseline: 1.0027x; 1.0027x over previous
"""Enformer-style relative-position attention (nn_Attention_27925877358942) for
8 Trainium2 NeuronCores.

Contract: kernel(**inputs) takes the FULL unsharded inputs (keys as in
setup_inputs()) and returns the full [1, 4096, 1536] float32 output.

Sharding: one head per core (8 heads / 8 cores). Host precomputes the
deterministic positional-feature table and x^T in fp16, slices per-head
weights, runs the SPMD Bass kernel via run_bass_kernel_spmd, and sums the
per-head output projections (+ b_out).

Device pipeline per core (head h), N=4096, d=64:
  - q,k projections on PE; stored fp8e4 in [32,2,*] DoubleRow layout
    (contraction d=64 packed as 32 partitions x 2) so the big matmuls run at
    0.5 cycles/row. r^T = (pos @ Wrelk_h)^T table in the same layout.
  - per query tile I: window logits L[di,c] = (q_i+bp).r[t0+c] (PE DoubleRow)
  - relative_shift via DRAM roundtrip: sheared strided read
      shr[di, j] = em[di, 127-di+j] (partition stride = rowpitch-1 elements)
  - two per-tile modes balance the ACT engine against the DMA engines:
    A-mode (fused exp): window LOGITS copied to SBUF bf16 (DVE/Pool),
      sheared in bf16; content logits k.qc accumulate in PSUM and the
      shifted rel logits are ADDED into the same PSUM by a plain matmul
      with identity as the moving operand (lhsT=shr block -> +=shr^T);
      a single ACT Exp pass produces p^T directly.
    B-mode (split exp): window logits Exp'd on ACT straight to fp8e4
      (bias -3.75 keeps values in fp8 range; row normalization cancels it),
      sheared in fp8 (half the DMA bytes); content logits Exp'd separately;
      p^T = exp(content)^T * transpose(shr) (PE transpose + DVE multiply).
  - O = pT.T @ [v|1] accumulated in PSUM; row-normalize the [128,64] o tile
    (cheap) before the W_out projection; Pool engine copies the projected
    output out of PSUM; host sums partials over heads.

This walrus build accepts at most ONE sync wait per instruction, so after
Tile scheduling every multi-wait instruction is split by inserting
wait-carrying NoOps just before it on the same engine (split_multi_waits),
and the Tile tail drain is built with the same constraint.
"""


_DRAIN_PATCHED = [False]


def _patch_tile_drain():
    if _DRAIN_PATCHED[0]:
        return
    _DRAIN_PATCHED[0] = True
    import concourse.tile as tile_mod
    from concourse.vector_clock import ScopedClock

    MAX_WAITS = 1

    def _drain_and_barrier(self, tick_clock, wait_clock):
        nc = self.nc
        drain_inst = nc.sync.drain()
        wait_clock.add_sem_waits(drain_inst.ins, ScopedClock({None: tick_clock.global_clock}))
        si = drain_inst.ins.sync_info
        waits = list(si.on_wait) if si is not None and si.on_wait else []
        if len(waits) > MAX_WAITS:
            si.on_wait = waits[:MAX_WAITS]
            rest = waits[MAX_WAITS:]
            import concourse.mybir as _mb
            for i in range(0, len(rest), MAX_WAITS):
                extra = nc.sync.drain()
                esi = extra.ins.sync_info
                if esi is None:
                    extra.ins.sync_info = _mb.SyncInfo(on_wait=rest[i:i + MAX_WAITS], on_update=[])
                else:
                    esi.on_wait = rest[i:i + MAX_WAITS]
        nc.all_engine_barrier()
        assert self.sems is not None
        popped = nc._tile_sem_poison_stack.pop()
        assert popped is self._sem_poison
        nc.clear_and_free_semaphores(list(self.sems.allocated().values()))
        nc.all_engine_barrier()

    tile_mod.TileContext._drain_and_barrier = _drain_and_barrier


def split_multi_waits(nc):
    """This walrus build allows at most ONE sync wait per instruction.
    Move extra waits onto InstNoOp carriers inserted just before, on the
    same engine queue (sequencers execute in order, so semantics hold)."""
    import concourse.mybir as mb
    n_split = 0
    for fn in nc.m.functions:
        for bb in fn.blocks:
            insts = list(bb.instructions)
            out = []
            for inst in insts:
                si = inst.sync_info
                waits = list(si.on_wait) if si is not None and si.on_wait else []
                if len(waits) > 1:
                    for w in waits[:-1]:
                        n_split += 1
                        nop = mb.InstNoOp(
                            name=f"waitsplit-{n_split}",
                            engine=inst.engine,
                            sync_info=mb.SyncInfo(on_wait=[w], on_update=[]),
                        )
                        out.append(nop)
                    si.on_wait = [waits[-1]]
                out.append(inst)
            if len(out) != len(insts):
                bb.instructions[:] = out
    return n_split


import math
from contextlib import ExitStack

import numpy as np

import concourse.bass as bass
import concourse.tile as tile
from concourse import mybir
from concourse.bass import ts, ds
from concourse.masks import make_identity

F32 = mybir.dt.float32
BF16 = mybir.dt.bfloat16
FP16 = mybir.dt.float16
FP8 = mybir.dt.float8e4
AF = mybir.ActivationFunctionType
DR = mybir.MatmulPerfMode.DoubleRow

DIM = 1536
H = 8
D = 64

# ---- tunables ----
N_A_PAIRS = 7          # of 16 q-tile pairs run in A-mode (fused exp, bf16 shear)
FP8_DR_CONTENT = True  # content logits matmul in fp8e4 DoubleRow
FP8_DR_EM = True       # rel-window logits matmul in fp8e4 DoubleRow
EM_SHIFT = 3.75        # B-mode: exp(rel - EM_SHIFT) to fit fp8e4 range
B_CAST_READ = False    # B-mode: cast the shear read fp8->bf16 via SWDGE
A_COPY_POOL_MOD = 3    # A-mode em-copy chunks sent to Pool every k-th chunk


def a_pair_set(n_pairs, n_a):
    if not n_a:
        return set()
    return {min(n_pairs - 1, round(i * n_pairs / n_a)) for i in range(n_a)}


def build(N, split_waits=True):
    Q = N // 128           # query tiles
    NJ = N // 128          # key tiles
    PW = 2 * N             # padded positional width (2n-1 real cols + 1 pad)
    WN = N + 128           # rel window width per q-tile (incl. 1 pad col)
    KD = DIM // 128        # contraction tiles for projections

    a_pairs = a_pair_set(Q // 2, N_A_PAIRS)
    n_a = 2 * len(a_pairs)
    n_b = Q - n_a

    nc = bass.Bass("TRN2", target_bir_lowering=False, debug=False)

    xT_d = nc.dram_tensor("xT", [DIM, N], FP16, kind="ExternalInput")
    posT_d = nc.dram_tensor("posT", [192, PW], FP16, kind="ExternalInput")
    wq_d = nc.dram_tensor("wq", [DIM, D], FP16, kind="ExternalInput")
    wk_d = nc.dram_tensor("wk", [DIM, D], FP16, kind="ExternalInput")
    wv_d = nc.dram_tensor("wv", [DIM, D], FP16, kind="ExternalInput")
    wrk_d = nc.dram_tensor("wrk", [192, D], FP16, kind="ExternalInput")
    wo_d = nc.dram_tensor("wo", [D, DIM], BF16, kind="ExternalInput")
    bc_d = nc.dram_tensor("bc", [D, 1], F32, kind="ExternalInput")
    bp_d = nc.dram_tensor("bp", [D, 1], F32, kind="ExternalInput")
    out_d = nc.dram_tensor("out", [N, DIM], FP16, kind="ExternalOutput")
    em16_d = em8_d = None
    if n_a:
        em16_d = nc.dram_tensor("em16_scratch", [n_a * 128, WN], BF16, kind="Internal")
    if n_b:
        em8_d = nc.dram_tensor("em8_scratch", [n_b * 128, WN], FP8, kind="Internal")

    scale = D ** -0.5

    with tile.TileContext(nc) as tc, ExitStack() as ctx:
        consts = ctx.enter_context(tc.tile_pool(name="consts", bufs=1))
        persist = ctx.enter_context(tc.tile_pool(name="persist", bufs=1))

        # ---- constants ----
        ident = consts.tile([128, 128], BF16, tag="ident")
        make_identity(nc, ident[:])
        bc_sb = consts.tile([D, 1], F32, tag="bc")
        nc.sync.dma_start(out=bc_sb[:], in_=bc_d.ap())
        bp_sb = consts.tile([D, 1], F32, tag="bp")
        nc.sync.dma_start(out=bp_sb[:], in_=bp_d.ap())
        wo_sb = consts.tile([D, DIM], BF16, tag="wo")
        nc.sync.dma_start(out=wo_sb[:], in_=wo_d.ap())
        negc_sb = consts.tile([128, 1], F32, tag="negc")
        nc.vector.memset(negc_sb[:], -EM_SHIFT)

        wqk_sb = consts.tile([128, KD, 2 * D], FP16, tag="wqk")
        wv_sb = consts.tile([128, KD, D], FP16, tag="wv")
        nc.sync.dma_start(out=wqk_sb[:, :, 0:D],
                          in_=wq_d.ap().rearrange("(t p) c -> p t c", p=128))
        nc.sync.dma_start(out=wqk_sb[:, :, D:2 * D],
                          in_=wk_d.ap().rearrange("(t p) c -> p t c", p=128))
        nc.sync.dma_start(out=wv_sb[:],
                          in_=wv_d.ap().rearrange("(t p) c -> p t c", p=128))
        wrk_sb = consts.tile([96, 2, D], FP16, tag="wrk")
        for u in range(2):
            nc.sync.dma_start(out=wrk_sb[:, u, :], in_=wrk_d[ts(u, 96), :])

        # ---- persistent activations ----
        # fp8 DoubleRow layout: contraction d = u*32+p on [32 partitions, 2].
        # qp | qc | k packed along the free axis of one [32, 2, 3N] tile.
        any_fp8 = FP8_DR_EM or FP8_DR_CONTENT
        if any_fp8:
            qpk8 = persist.tile([32, 2, 3 * N], FP8, tag="qpk8")
        if FP8_DR_EM:
            rT8 = persist.tile([32, 2, PW], FP8, tag="rT8")
        else:
            qpT = persist.tile([D, N], FP16, tag="qpT")
            rT = persist.tile([D, PW], FP16, tag="rT")
        if not FP8_DR_CONTENT:
            qcT = persist.tile([D, N], FP16, tag="qcT")
            kT = persist.tile([D, N], FP16, tag="kT")
        vext = persist.tile([128, NJ * (D + 1)], BF16, tag="vext")

        # ---- phases 1-2: rel-k table + projections ----
        with tc.tile_pool(name="stream", bufs=1) as stream, \
             tc.tile_pool(name="prep_psum", bufs=2, space="PSUM") as prep_psum:
            # rel-k table first: independent of x, runs during the xT load
            pall = stream.tile([96, 2, PW], FP16, tag="pall")
            nc.sync.dma_start(out=pall[:, 0, :], in_=posT_d[0:96, :])
            nc.sync.dma_start(out=pall[:, 1, :], in_=posT_d[96:192, :])
            for rc in reversed(range(PW // 512)):
                c0 = rc * 512
                pc = pall[:, :, ds(c0, 512)]
                ps_r = prep_psum.tile([D, 512], F32, tag="ps_qk")
                for u in range(2):
                    nc.tensor.matmul(
                        ps_r[:], wrk_sb[:, u, :], pc[:, u, :],
                        start=(u == 0), stop=(u == 1),
                    )
                if FP8_DR_EM:
                    for u in range(2):
                        nc.scalar.copy(out=rT8[:, u, ds(c0, 512)],
                                       in_=ps_r[ts(u, 32), :])
                else:
                    nc.scalar.copy(out=rT[:, ds(c0, 512)], in_=ps_r[:])

            xall = stream.tile([128, KD, N], FP16, tag="xall")
            xT_v = xT_d.ap().rearrange("(t p) n -> p t n", p=128)
            for oct_ in range(8):
                h0 = oct_ * (N // 8)
                nc.sync.dma_start(
                    out=xall[:, :, ds(h0, N // 8)],
                    in_=xT_v[:, :, ds(h0, N // 8)],
                )
            for ic in range(N // 512):
                i0 = ic * 512
                xc = xall[:, :, ds(i0, 512)]
                ps_qk = prep_psum.tile([128, 512], F32, tag="ps_qk")
                for kd in range(KD):
                    nc.tensor.matmul(
                        ps_qk[:], wqk_sb[:, kd, :], xc[:, kd, :],
                        start=(kd == 0), stop=(kd == KD - 1),
                    )
                if FP8_DR_EM:
                    for u in range(2):
                        nc.scalar.activation(
                            out=qpk8[:, u, ds(i0, 512)],
                            in_=ps_qk[ts(u, 32), :], func=AF.Identity,
                            bias=bp_sb[ts(u, 32), :], scale=scale,
                        )
                else:
                    nc.scalar.activation(
                        out=qpT[:, ds(i0, 512)], in_=ps_qk[0:D, :], func=AF.Identity,
                        bias=bp_sb[:], scale=scale,
                    )
                if FP8_DR_CONTENT:
                    for u in range(2):
                        nc.scalar.activation(
                            out=qpk8[:, u, ds(N + i0, 512)],
                            in_=ps_qk[ts(u, 32), :], func=AF.Identity,
                            bias=bc_sb[ts(u, 32), :], scale=scale,
                        )
                        nc.scalar.copy(
                            out=qpk8[:, u, ds(2 * N + i0, 512)],
                            in_=ps_qk[D + u * 32:D + (u + 1) * 32, :],
                        )
                else:
                    nc.scalar.activation(
                        out=qcT[:, ds(i0, 512)], in_=ps_qk[0:D, :], func=AF.Identity,
                        bias=bc_sb[:], scale=scale,
                    )
                    nc.scalar.copy(out=kT[:, ds(i0, 512)], in_=ps_qk[D:2 * D, :])
                for isb in range(4):
                    J = ic * 4 + isb
                    ps_v = prep_psum.tile([128, D], F32, tag="ps_v")
                    for kd in range(KD):
                        nc.tensor.matmul(
                            ps_v[:], xc[:, kd, ts(isb, 128)], wv_sb[:, kd, :],
                            start=(kd == 0), stop=(kd == KD - 1),
                        )
                    nc.scalar.copy(out=vext[:, ds(J * (D + 1), D)], in_=ps_v[:])
                    nc.vector.memset(vext[:, ds(J * (D + 1) + D, 1)], 1.0)

        def em_matmul(ps_slice, i0, w0, sw):
            """rel-window logits for q-tile at i0, window cols [w0, w0+sw)."""
            if FP8_DR_EM:
                nc.tensor.matmul(
                    ps_slice, qpk8[:, :, ds(i0, 128)], rT8[:, :, ds(w0, sw)],
                    perf_mode=DR, start=True, stop=True,
                )
            else:
                nc.tensor.matmul(
                    ps_slice, qpT[:, ds(i0, 128)], rT[:, ds(w0, sw)],
                    start=True, stop=True,
                )

        def content_matmul(ps_slice, J, q0, width, start, stop):
            """content logits^T: keys J-tile x query cols [q0, q0+width)."""
            if FP8_DR_CONTENT:
                nc.tensor.matmul(
                    ps_slice, qpk8[:, :, ds(2 * N + J * 128, 128)],
                    qpk8[:, :, ds(N + q0, width)],
                    perf_mode=DR, start=start, stop=stop,
                )
            else:
                nc.tensor.matmul(
                    ps_slice, kT[:, ts(J, 128)], qcT[:, ds(q0, width)],
                    start=start, stop=stop,
                )

        # ---- phase 3: main loop, q-tiles in pairs ----
        work = ctx.enter_context(tc.tile_pool(name="work", bufs=2))
        ect_pool = ctx.enter_context(tc.tile_pool(name="ect", bufs=3))
        sm = ctx.enter_context(tc.tile_pool(name="sm", bufs=3))
        ppool_m = ctx.enter_context(tc.tile_pool(name="ppool_m", bufs=2, space="PSUM"))
        ppool_ct = ctx.enter_context(tc.tile_pool(name="ppool_ct", bufs=2, space="PSUM"))
        ppool_st = ctx.enter_context(tc.tile_pool(name="ppool_st", bufs=1, space="PSUM"))
        ppool_epi = ctx.enter_context(tc.tile_pool(name="ppool_epi", bufs=1, space="PSUM"))
        wshear_a = ctx.enter_context(tc.tile_pool(name="wshear_a", bufs=3)) if n_a else None
        wshear_b = ctx.enter_context(tc.tile_pool(name="wshear_b", bufs=4)) if n_b else None

        b_shr_t = BF16 if B_CAST_READ else FP8

        a_slot = 0
        b_slot = 0
        n_copy = 0
        for g in range(Q // 2):
            i0g = g * 256
            mode_a = g in a_pairs
            shr_pair = []
            for q in range(2):
                I = 2 * g + q
                i0 = I * 128
                t0 = N - 1 - i0 - 127

                if mode_a:
                    em_sb = wshear_a.tile([128, WN], BF16, tag="em16")
                else:
                    em_sb = wshear_b.tile([128, WN], FP8, tag="em8")
                n_full = (WN - 128) // 1024
                chunks = [(c * 1024, 1024) for c in range(n_full)]
                chunks.append((n_full * 1024, WN - 1 - n_full * 1024))
                for (c0, cw) in chunks:
                    ps = ppool_m.tile([128, 1024], F32, tag="ps_m")
                    for s0 in range(0, cw, 512):
                        sw = min(512, cw - s0)
                        em_matmul(ps[:, ds(s0, sw)], i0, t0 + c0 + s0, sw)
                    if mode_a:
                        n_copy += 1
                        eng = nc.gpsimd if (n_copy % A_COPY_POOL_MOD == 0) else nc.vector
                        eng.tensor_copy(em_sb[:, ds(c0, cw)], ps[:, 0:cw])
                    else:
                        nc.scalar.activation(
                            out=em_sb[:, ds(c0, cw)], in_=ps[:, 0:cw], func=AF.Exp,
                            bias=negc_sb[:],
                        )
                if mode_a:
                    slot, em_dst = a_slot, em16_d
                    a_slot += 1
                    shr_sb = wshear_a.tile([128, N], BF16, tag="shr16")
                else:
                    slot, em_dst = b_slot, em8_d
                    b_slot += 1
                    shr_sb = wshear_b.tile([128, N], b_shr_t, tag="shr")
                nc.sync.dma_start(out=em_dst[ds(slot * 128, 128), 0:WN - 1],
                                  in_=em_sb[:, 0:WN - 1])
                shear_ap = bass.AP(em_dst, slot * 128 * WN + 127,
                                   [[WN - 1, 128], [1, N]])
                if (not mode_a) and B_CAST_READ:
                    nc.gpsimd.dma_start(out=shr_sb[:], in_=shear_ap)
                else:
                    nc.sync.dma_start(out=shr_sb[:], in_=shear_ap)
                shr_pair.append(shr_sb)

            # pT[dj, J*256 + q*128 + di] = p^T for the pair
            pT_sb = work.tile([128, NJ * 256], BF16, tag="pT")
            if mode_a:
                # content + shifted rel accumulate in PSUM; single Exp pass
                for Jg in range(NJ // 2):
                    ps = ppool_ct.tile([128, 512], F32, tag="ps_ct")
                    for u in range(2):
                        J = Jg * 2 + u
                        for q in range(2):
                            sl = ps[:, ds(u * 256 + q * 128, 128)]
                            content_matmul(sl, J, i0g + q * 128, 128,
                                           start=True, stop=False)
                            nc.tensor.matmul(
                                sl, shr_pair[q][:, ts(J, 128)], ident[:],
                                start=False, stop=True,
                            )
                    nc.scalar.activation(
                        out=pT_sb[:, ds(Jg * 512, 512)], in_=ps[:], func=AF.Exp,
                    )
            else:
                # split exp: ecT = exp(content^T - 4); pT = ecT * shr^T
                for Jg2 in range(NJ // 4):
                    ecT_sb = ect_pool.tile([128, 1024], BF16, tag="ecT")
                    for hh in range(2):
                        Jg = Jg2 * 2 + hh
                        ps = ppool_ct.tile([128, 512], F32, tag="ps_ct")
                        for u in range(2):
                            J = Jg * 2 + u
                            content_matmul(ps[:, ts(u, 256)], J, i0g, 256,
                                           start=True, stop=True)
                        nc.scalar.activation(
                            out=ecT_sb[:, ds(hh * 512, 512)], in_=ps[:], func=AF.Exp,
                        )
                    ps_t = ppool_st.tile([128, 1024], b_shr_t, tag="ps_st")
                    for u in range(4):
                        J = Jg2 * 4 + u
                        for q in range(2):
                            nc.tensor.transpose(
                                ps_t[:, ds(u * 256 + q * 128, 128)],
                                shr_pair[q][:, ts(J, 128)], ident[:],
                            )
                    nc.vector.tensor_mul(
                        pT_sb[:, ds(Jg2 * 1024, 1024)], ecT_sb[:], ps_t[:]
                    )

            # PV + epilogue per q-tile
            for q in range(2):
                i0 = i0g + q * 128
                ps_o = ppool_epi.tile([128, D + 1], F32, tag="ps_epi")
                for J in range(NJ):
                    nc.tensor.matmul(
                        ps_o[:], pT_sb[:, ds(J * 256 + q * 128, 128)],
                        vext[:, ds(J * (D + 1), D + 1)],
                        start=(J == 0), stop=(J == NJ - 1),
                    )
                rc_sb = sm.tile([128, 1], F32, tag="rc")
                nc.vector.reciprocal(out=rc_sb[:], in_=ps_o[:, D:D + 1])
                o_sb = sm.tile([128, D], BF16, tag="o")
                nc.vector.tensor_scalar_mul(o_sb[:], ps_o[:, 0:D], rc_sb[:])
                ps_ot = ppool_epi.tile([D, 128], BF16, tag="ps_epi")
                nc.tensor.transpose(ps_ot[:], o_sb[:], ident[:])
                otT_sb = sm.tile([D, 128], BF16, tag="otT")
                nc.vector.tensor_copy(otT_sb[:], ps_ot[:])
                out_sb = work.tile([128, DIM], FP16, tag="out")
                for w in range(DIM // 512):
                    ps_op = ppool_epi.tile([128, 512], F32, tag="ps_epi")
                    nc.tensor.matmul(
                        ps_op[:], otT_sb[:], wo_sb[:, ts(w, 512)],
                        start=True, stop=True,
                    )
                    nc.gpsimd.tensor_copy(out_sb[:, ts(w, 512)], ps_op[:])
                nc.sync.dma_start(out=out_d[ds(i0, 128), :], in_=out_sb[:])

    if split_waits:
        _patch_tile_drain()
        split_multi_waits(nc)
    return nc


# ---------------- host side ----------------

def get_positional_embed_np(seq_len, feature_size):
    distances = np.arange(-seq_len + 1, seq_len)
    nb = feature_size // 2
    pow_rate = math.exp(math.log(seq_len + 1) / nb)
    center_widths = np.power(np.float32(pow_rate), np.arange(1, nb + 1, dtype=np.float32)) - 1.0
    emb = (center_widths[None, :] > np.abs(distances)[:, None]).astype(np.float32)
    signed = np.sign(distances).astype(np.float32)[:, None] * emb
    return np.concatenate([emb, signed], axis=-1)  # [2n-1, F]


def make_in_maps(x, W_q, W_k, W_v, W_rel_k, W_out, rel_content_bias, rel_pos_bias):
    B, N, _ = np.asarray(x).shape
    PW = 2 * N
    f16 = np.float16
    import ml_dtypes
    bf16 = ml_dtypes.bfloat16
    xT = np.ascontiguousarray(np.asarray(x[0], np.float32).T).astype(f16)
    pos = get_positional_embed_np(N, np.asarray(W_rel_k).shape[0])
    posT = np.zeros((192, PW), np.float32)
    posT[:, : 2 * N - 1] = pos.T
    posT = posT.astype(f16)
    in_maps = []
    for h in range(H):
        sl = slice(h * D, (h + 1) * D)
        in_maps.append({
            "xT": xT,
            "posT": posT,
            "wq": np.ascontiguousarray(np.asarray(W_q)[:, sl]).astype(f16),
            "wk": np.ascontiguousarray(np.asarray(W_k)[:, sl]).astype(f16),
            "wv": np.ascontiguousarray(np.asarray(W_v)[:, sl]).astype(f16),
            "wrk": np.ascontiguousarray(np.asarray(W_rel_k)[:, sl]).astype(f16),
            "wo": np.ascontiguousarray(np.asarray(W_out)[sl, :]).astype(bf16),
            "bc": np.ascontiguousarray(
                np.asarray(rel_content_bias, np.float32)[0, h, 0, :].reshape(D, 1)),
            "bp": np.ascontiguousarray(
                np.asarray(rel_pos_bias, np.float32)[0, h, 0, :].reshape(D, 1)),
        })
    return in_maps


def combine_outputs(results, b_out):
    acc = None
    for r in results:
        p = r["out"].astype(np.float32)
        acc = p if acc is None else acc + p
    acc = acc + np.asarray(b_out, np.float32)[None, :]
    return acc[None]  # [1, N, DIM]


# ---------------- entry point ----------------

_NC_CACHE = {}


def kernel(x, W_q, W_k, W_v, W_rel_k, W_out, b_out,
           rel_content_bias, rel_pos_bias):
    """Full-input entry: shards per head across 8 NeuronCores, returns the
    full [1, N, 1536] float32 output."""
    from concourse import bass_utils

    x = np.asarray(x)
    N = x.shape[1]
    if N not in _NC_CACHE:
        _NC_CACHE[N] = build(N)
    nc = _NC_CACHE[N]
    in_maps = make_in_maps(x, W_q, W_k, W_v, W_rel_k, W_out,
                           rel_content_bias, rel_pos_bias)
    res = bass_utils.run_bass_kernel_spmd(nc, in_maps, core_ids=list(range(H)))
    return combine_outputs(res.results, b_out).astype(np.float32)
